# revision 8
# baseline (speedup 1.0000x reference)
"""Trainium2 Bass kernel for FNO1d (B=64, N=8192, W=64, MODES=16, 4 layers).

Pure data-parallel over batch: 8 elements per NeuronCore on 8 cores. Per core,
elements form 4 pairs (2x64 channels -> 128 partitions). rfft/irfft are
replaced by 16-mode DFT matmuls. All activations/weights are fp16 in SBUF
(accumulation in fp32 PSUM); biases are folded into matmuls (fc0 via a ones
row, layer cb via an extra ones-row of the inverse basis).
"""
import sys
import numpy as np

sys.path.insert(0, "/opt/trn_rl_repo")

import concourse.bacc as bacc
import concourse.mybir as mybir
import concourse.tile as tile
from concourse.bass_utils import run_bass_kernel_spmd

F16 = mybir.dt.float16
F32 = mybir.dt.float32
GELU = mybir.ActivationFunctionType.Gelu
COPY = mybir.ActivationFunctionType.Copy

B, N, W, MODES, L = 64, 8192, 64, 16, 4
NCORES = 8
E = B // NCORES          # 8 elems per core
NP = E // 2              # 4 pairs
K2 = 2 * MODES           # 32 interleaved (cos,sin) rows
CH = 512
GRP = 1024
NGRP = N // GRP          # 8
NA = N // 128            # 64 transpose chunks
SL = 2048                # transpose slice width
NSL = N // SL            # 4 slices
APS = SL // 128          # a-chunks per slice = 16

_cache = {}


def _build():
    if "nc" in _cache:
        return _cache["nc"]
    nc = bacc.Bacc("TRN2", target_bir_lowering=False, debug=False,
                   num_devices=NCORES)

    def din(name, shape, dt):
        return nc.dram_tensor(name, shape, dt, kind="ExternalInput").ap()

    x8 = din("x8", [E, N], F16)
    t8 = din("t8", [E, N], F16)
    fb_d = din("fb", [128, NA, K2], F16)         # fb[p,a,kk] = Fb[128a+p, kk]
    ib_d = din("ib", [128, N], F16)              # rows 0:33 & 64:97 = [IB; ones]
    wm_d = din("wm", [64, L, MODES, 3, 64], F16)  # (i, l, k, {wr, wi, -wr}, o)
    cwt_d = din("cwt", [128, L, 128], F16)       # block-diag [cw.T 0; 0 cw.T]
    w03_d = din("w03", [6, 128], F16)            # block-diag fc0 (w0,w1,b)x2
    fc1w_d = din("fc1w", [128, 128], F16)        # both bands = fc1_w
    fc2w_d = din("fc2w", [128, 1], F16)
    cbr_d = din("cbrep", [128, L * 512], F16)    # rows 32/96: tile(cb[l], 8)
    fc1b_d = din("fc1b", [128, 1], F32)
    fc2b_d = din("fc2b", [128, 1], F32)
    idn_d = din("idn", [64, 64], F16)
    out_d = nc.dram_tensor("out", [E, N], F32, kind="ExternalOutput").ap()

    with tile.TileContext(nc) as tc:
        import contextlib
        with contextlib.ExitStack() as ctx:
            const = ctx.enter_context(tc.tile_pool(name="const", bufs=1))
            hpool = ctx.enter_context(tc.tile_pool(name="h", bufs=1))
            tp = ctx.enter_context(tc.tile_pool(name="tp", bufs=3))
            small = ctx.enter_context(tc.tile_pool(name="small", bufs=2))
            wmp = ctx.enter_context(tc.tile_pool(name="wmp", bufs=1))
            gpool = ctx.enter_context(tc.tile_pool(name="g", bufs=2))
            opool = ctx.enter_context(tc.tile_pool(name="o", bufs=2))
            grid = ctx.enter_context(tc.tile_pool(name="grid", bufs=3))
            ps = ctx.enter_context(tc.tile_pool(name="ps", bufs=1, space="PSUM"))
            psz = ctx.enter_context(tc.tile_pool(name="psz", bufs=2, space="PSUM"))

            # ---- constants ----
            fb = const.tile([128, NA * K2], F16, tag="fb")
            nc.sync.dma_start(out=fb[:], in_=fb_d.rearrange("p a k -> p (a k)"))
            ib = const.tile([128, N], F16, tag="ib")
            nc.sync.dma_start(out=ib[:], in_=ib_d)
            cwt = const.tile([128, L * 128], F16, tag="cwt")
            nc.sync.dma_start(out=cwt[:], in_=cwt_d.rearrange("p l o -> p (l o)"))
            w03 = const.tile([6, 128], F16, tag="w03")
            nc.sync.dma_start(out=w03[:], in_=w03_d)
            fc1w = const.tile([128, 128], F16, tag="fc1w")
            nc.sync.dma_start(out=fc1w[:], in_=fc1w_d)
            fc2w = const.tile([128, 1], F16, tag="fc2w")
            nc.sync.dma_start(out=fc2w[:], in_=fc2w_d)
            cbrep = const.tile([128, L * 512], F16, tag="cbrep")
            nc.sync.dma_start(out=cbrep[:], in_=cbr_d)
            fc1b = const.tile([128, 1], F32, tag="fc1b")
            nc.sync.dma_start(out=fc1b[:], in_=fc1b_d)
            fc2b = const.tile([128, 1], F32, tag="fc2b")
            nc.sync.dma_start(out=fc2b[:], in_=fc2b_d)
            idn = const.tile([64, 64], F16, tag="idn")
            nc.sync.dma_start(out=idn[:], in_=idn_d)

            # persistent per-pair activations [128, N] fp16
            h = [hpool.tile([128, N], F16, tag=f"h{p}", name=f"h{p}")
                 for p in range(NP)]

            # ---- fc0 (block-diag K=6, bias via ones rows) ----
            for p in range(NP):
                for g in range(NGRP):
                    zp = psz.tile([128, GRP], F32, tag="z")
                    gt = grid.tile([6, GRP], F16, tag="grid")
                    nc.vector.memset(gt[:], 1.0)
                    eA, eB = 2 * p, 2 * p + 1
                    gw = slice(g * GRP, (g + 1) * GRP)
                    nc.sync.dma_start(out=gt[0:1, :], in_=x8[eA:eA + 1, gw])
                    nc.sync.dma_start(out=gt[1:2, :], in_=t8[eA:eA + 1, gw])
                    nc.sync.dma_start(out=gt[3:4, :], in_=x8[eB:eB + 1, gw])
                    nc.sync.dma_start(out=gt[4:5, :], in_=t8[eB:eB + 1, gw])
                    for c2 in range(GRP // CH):
                        sl = slice(c2 * CH, (c2 + 1) * CH)
                        nc.tensor.matmul(zp[:, sl], w03[:], gt[:, sl],
                                         start=True, stop=True)
                    nc.scalar.activation(h[p][:, gw], zp[:], COPY)

            # ---- FNO layers ----
            for l in range(L):
                wml = wmp.tile([64, MODES * 3 * 64], F16, tag="wml")
                nc.sync.dma_start(out=wml[:], in_=wm_d[:, l].rearrange("p b c d -> p (b c d)"))

                # transpose h slices + forward FT (accumulate over n chunks)
                ftp = ps.tile([K2, 4 * 128], F32, tag="ft")
                for p in range(NP):
                    for q in range(NSL):
                        ht = tp.tile([128, SL], F16, tag="ht")
                        nc.sync.dma_start(out=ht[:].rearrange("p (a c) -> p a c", a=APS),
                                          in_=h[p][:, q * SL:(q + 1) * SL],
                                          transpose=True)
                        for aa in range(APS):
                            a = q * APS + aa
                            nc.tensor.matmul(ftp[:, 128 * p:128 * (p + 1)],
                                             fb[:, a * K2:(a + 1) * K2],
                                             ht[:, aa * 128:(aa + 1) * 128],
                                             start=(a == 0), stop=(a == NA - 1))
                ftsb = small.tile([K2, 512], F16, tag="ftsb")
                nc.vector.tensor_copy(ftsb[:], ftp[:])

                # FTI[i, 32 e + kk] via 2 stream transposes (i-halves)
                fti = small.tile([64, 256], F16, tag="fti")
                ftsb3 = ftsb[:].rearrange("p (e c) -> p e c", e=8)
                for hh in range(2):
                    nc.vector.transpose(fti[32 * hh:32 * hh + 32, :],
                                        ftsb3[:, :, 32 * hh:32 * hh + 32])

                # mode mix -> mmp[o, 8*(2k+s') + e]
                mmp = ps.tile([64, 256], F32, tag="mm")
                for k in range(MODES):
                    wb = (k * 3) * 64
                    wr = wml[:, wb:wb + 64]
                    wi = wml[:, wb + 64:wb + 128]
                    nwr = wml[:, wb + 128:wb + 192]
                    ftc = fti[:, 2 * k::K2]
                    fts = fti[:, 2 * k + 1::K2]
                    nc.tensor.matmul(mmp[:, 16 * k:16 * k + 8], wr, ftc,
                                     start=True, stop=False)
                    nc.tensor.matmul(mmp[:, 16 * k:16 * k + 8], wi, fts,
                                     start=False, stop=True)
                    nc.tensor.matmul(mmp[:, 16 * k + 8:16 * k + 16], wi, ftc,
                                     start=True, stop=False)
                    nc.tensor.matmul(mmp[:, 16 * k + 8:16 * k + 16], nwr, fts,
                                     start=False, stop=True)
                mmsb = small.tile([64, 256], F16, tag="mmsb")
                nc.vector.tensor_copy(mmsb[:], mmp[:])

                # MT[band + kk, 64 e + o] via 16 PE transposes; row 32/96 = cb
                mtp = ps.tile([128, 512], F16, tag="mtops")
                for e in range(8):
                    src = mmsb[:, e::8]   # [64 o, 32 kk]
                    for band in range(2):
                        nc.tensor.transpose(mtp[64 * band:64 * band + 32, 64 * e:64 * e + 64],
                                            src, idn[:], tile_position=(0, 64 * band))
                mt = small.tile([128, 512], F16, tag="mtsb")
                nc.vector.tensor_copy(mt[:], mtp[:])
                nc.vector.tensor_copy(mt[32:33, :], cbrep[32:33, l * 512:(l + 1) * 512])
                nc.vector.tensor_copy(mt[96:97, :], cbrep[96:97, l * 512:(l + 1) * 512])

                # inverse FT (K=33 incl. bias row) + pointwise (block-diag) + act
                act = GELU if l < L - 1 else COPY
                for p in range(NP):
                    for g in range(NGRP):
                        zp = psz.tile([128, GRP], F32, tag="z")
                        for c2 in range(GRP // CH):
                            c0 = g * GRP + c2 * CH
                            sl = slice(c2 * CH, (c2 + 1) * CH)
                            nc.tensor.matmul(zp[:, sl], cwt[:, l * 128:(l + 1) * 128],
                                             h[p][:, c0:c0 + CH], start=True, stop=False)
                            nc.tensor.matmul(zp[0:64, sl],
                                             mt[0:33, (2 * p) * 64:(2 * p) * 64 + 64],
                                             ib[0:33, c0:c0 + CH], start=False, stop=True,
                                             tile_position=(0, 0))
                            nc.tensor.matmul(zp[64:128, sl],
                                             mt[64:97, (2 * p + 1) * 64:(2 * p + 1) * 64 + 64],
                                             ib[64:97, c0:c0 + CH], start=False, stop=True,
                                             tile_position=(64, 64))
                        nc.scalar.activation(h[p][:, g * GRP:(g + 1) * GRP], zp[:], act)

            # ---- fc1 -> gelu -> fc2 -> out ----
            for p in range(NP):
                for g in range(NGRP):
                    op = ps.tile([128, GRP], F32, tag="mtops")
                    for e2 in range(2):
                        gp = psz.tile([128, GRP], F32, tag="z")
                        for c2 in range(GRP // CH):
                            c0 = g * GRP + c2 * CH
                            nc.tensor.matmul(gp[:, c2 * CH:(c2 + 1) * CH],
                                             fc1w[64 * e2:64 * e2 + 64, :],
                                             h[p][64 * e2:64 * e2 + 64, c0:c0 + CH],
                                             start=True, stop=True)
                        gsb = gpool.tile([128, GRP], F16, tag="gsb")
                        nc.scalar.activation(gsb[:], gp[:], GELU, bias=fc1b[:])
                        for c2 in range(GRP // CH):
                            sl = slice(c2 * CH, (c2 + 1) * CH)
                            nc.tensor.matmul(op[32 * e2:32 * e2 + 1, sl], fc2w[:],
                                             gsb[:, sl], start=True, stop=True,
                                             tile_position=(0, 32 * e2))
                    osb = opool.tile([64, GRP], F32, tag="osb")
                    nc.vector.tensor_scalar_add(osb[:], op[0:64, :], fc2b[0:64, :])
                    for e2 in range(2):
                        e = 2 * p + e2
                        nc.sync.dma_start(out=out_d[e:e + 1, g * GRP:(g + 1) * GRP],
                                          in_=osb[32 * e2:32 * e2 + 1, :])

    nc.compile()
    _cache["nc"] = nc
    return nc


def _consts(fc0_w, fc0_b, sw_r, sw_i, cw, cb, fc1_w, fc1_b, fc2_w, fc2_b):
    f16 = np.float16
    n = np.arange(N)
    k = np.arange(MODES)
    ang = 2.0 * np.pi * np.outer(n, k) / N            # [N, MODES]
    Fb = np.empty((N, K2), np.float32)
    Fb[:, 0::2] = np.cos(ang)
    Fb[:, 1::2] = np.sin(ang)
    fb = np.ascontiguousarray(
        Fb.reshape(NA, 128, K2).transpose(1, 0, 2)).astype(f16)

    alpha = np.where(k == 0, 1.0, 2.0)
    IBr = np.empty((K2, N), np.float32)
    IBr[0::2, :] = (alpha[:, None] * np.cos(ang.T)) / N
    IBr[1::2, :] = -2.0 * np.sin(ang.T) / N
    ib = np.zeros((128, N), np.float32)
    ib[0:K2] = IBr
    ib[64:64 + K2] = IBr
    ib[K2, :] = 1.0
    ib[64 + K2, :] = 1.0
    ib = ib.astype(f16)

    wm = np.empty((64, L, MODES, 3, 64), np.float32)
    for l in range(L):
        for kk in range(MODES):
            wm[:, l, kk, 0, :] = sw_r[l, :, :, kk]
            wm[:, l, kk, 1, :] = sw_i[l, :, :, kk]
            wm[:, l, kk, 2, :] = -sw_r[l, :, :, kk]
    wm = wm.astype(f16)

    cwt = np.zeros((128, L, 128), np.float32)
    for l in range(L):
        cwt[0:64, l, 0:64] = cw[l].T
        cwt[64:128, l, 64:128] = cw[l].T

    cbrep = np.zeros((128, L * 512), np.float32)
    for l in range(L):
        cbrep[K2, l * 512:(l + 1) * 512] = np.tile(cb[l], 8)
        cbrep[64 + K2, l * 512:(l + 1) * 512] = np.tile(cb[l], 8)

    w03s = np.concatenate([fc0_w, fc0_b[None, :]], axis=0).astype(np.float32)
    w03 = np.zeros((6, 128), np.float32)
    w03[0:3, 0:64] = w03s
    w03[3:6, 64:128] = w03s
    return dict(
        fb=fb, ib=ib, wm=wm,
        cwt=cwt.astype(f16), cbrep=cbrep.astype(f16), w03=w03.astype(f16),
        fc1w=np.concatenate([fc1_w, fc1_w], axis=0).astype(f16),
        fc2w=fc2_w.astype(f16),
        fc1b=fc1_b.astype(np.float32).reshape(128, 1),
        fc2b=np.full((128, 1), np.float32(np.asarray(fc2_b).reshape(-1)[0])),
        idn=np.eye(64, dtype=f16),
    )


def kernel(x, t, fc0_w, fc0_b, sw_r, sw_i, cw, cb, fc1_w, fc1_b, fc2_w, fc2_b,
           _trace=False, _tmpdir=None):
    nc = _build()
    consts = _consts(np.asarray(fc0_w), np.asarray(fc0_b), np.asarray(sw_r),
                     np.asarray(sw_i), np.asarray(cw), np.asarray(cb),
                     np.asarray(fc1_w), np.asarray(fc1_b), np.asarray(fc2_w),
                     np.asarray(fc2_b))
    x = np.asarray(x, np.float32).reshape(B, N).astype(np.float16)
    t = np.asarray(t, np.float32).reshape(B, N).astype(np.float16)
    in_maps = []
    for c in range(NCORES):
        m = dict(consts)
        m["x8"] = np.ascontiguousarray(x[c * E:(c + 1) * E])
        m["t8"] = np.ascontiguousarray(t[c * E:(c + 1) * E])
        in_maps.append(m)
    res = run_bass_kernel_spmd(nc, in_maps, list(range(NCORES)),
                               trace=_trace, tmpdir=_tmpdir)
    out = np.concatenate([res.results[c]["out"] for c in range(NCORES)], axis=0)
    kernel.last_result = res
    return out.reshape(B, N, 1).astype(np.float32)


# revision 10
# speedup vs baseline: 1.3512x; 1.3512x over previous
"""Trainium2 Bass kernel for FNO1d (B=64, N=8192, W=64, MODES=16, 4 layers).

Pure data-parallel over batch: 8 elements per NeuronCore on 8 cores. Per core,
elements form 4 pairs (2x64 channels -> 128 partitions). rfft/irfft are
replaced by 16-mode DFT matmuls. All activations/weights are fp16 in SBUF
(accumulation in fp32 PSUM); biases are folded into matmuls (fc0 via a ones
row, layer cb via an extra ones-row of the inverse basis).
"""
import sys
import numpy as np

sys.path.insert(0, "/opt/trn_rl_repo")

import concourse.bacc as bacc
import concourse.mybir as mybir
import concourse.tile as tile
from concourse.bass_utils import run_bass_kernel_spmd

F16 = mybir.dt.float16
F32 = mybir.dt.float32
GELU = mybir.ActivationFunctionType.Gelu
COPY = mybir.ActivationFunctionType.Copy

B, N, W, MODES, L = 64, 8192, 64, 16, 4
NCORES = 8
E = B // NCORES          # 8 elems per core
NP = E // 2              # 4 pairs
K2 = 2 * MODES           # 32 interleaved (cos,sin) rows
CH = 512
GRP = 1024
NGRP = N // GRP          # 8
NA = N // 128            # 64 transpose chunks
SL = 2048                # transpose slice width
NSL = N // SL            # 4 slices
APS = SL // 128          # a-chunks per slice = 16

_cache = {}


def _build():
    if "nc" in _cache:
        return _cache["nc"]
    nc = bacc.Bacc("TRN2", target_bir_lowering=False, debug=False,
                   num_devices=NCORES)

    def din(name, shape, dt):
        return nc.dram_tensor(name, shape, dt, kind="ExternalInput").ap()

    xt_d = din("xt", [128, N], F16)      # rows 32p..32p+6 = (xA, tA, 1, xB, tB, 1)
    fb_d = din("fb", [128, NA, K2], F16)         # fb[p,a,kk] = Fb[128a+p, kk]
    ib_d = din("ib", [128, N], F16)              # rows 0:33 & 64:97 = [IB; ones]
    wm_d = din("wm", [64, L, MODES, 3, 64], F16)  # (i, l, k, {wr, wi, -wr}, o)
    cwt_d = din("cwt", [128, L, 128], F16)       # block-diag [cw.T 0; 0 cw.T]
    w03_d = din("w03", [128, 128], F16)          # fc0 block at rows 32p
    fc1w_d = din("fc1w", [128, 128], F16)        # both bands = fc1_w
    fc2w_d = din("fc2w", [128, 1], F16)
    cbr_d = din("cbrep", [128, L * 512], F16)    # rows 32/96: tile(cb[l], 8)
    fc1b_d = din("fc1b", [128, 1], F32)
    fc2b_d = din("fc2b", [128, 1], F32)
    idn_d = din("idn", [64, 64], F16)
    out_d = nc.dram_tensor("out", [E, N], F32, kind="ExternalOutput").ap()

    with tile.TileContext(nc) as tc:
        import contextlib
        with contextlib.ExitStack() as ctx:
            const = ctx.enter_context(tc.tile_pool(name="const", bufs=1))
            hpool = ctx.enter_context(tc.tile_pool(name="h", bufs=1))
            htp = ctx.enter_context(tc.tile_pool(name="htp", bufs=1))
            small = ctx.enter_context(tc.tile_pool(name="small", bufs=2))
            wmp = ctx.enter_context(tc.tile_pool(name="wmp", bufs=1))
            gpool = ctx.enter_context(tc.tile_pool(name="g", bufs=2))
            opool = ctx.enter_context(tc.tile_pool(name="o", bufs=2))
            ps = ctx.enter_context(tc.tile_pool(name="ps", bufs=1, space="PSUM"))
            psz = ctx.enter_context(tc.tile_pool(name="psz", bufs=2, space="PSUM"))

            # ---- constants ----
            fb = const.tile([128, NA * K2], F16, tag="fb")
            nc.sync.dma_start(out=fb[:], in_=fb_d.rearrange("p a k -> p (a k)"))
            ib = const.tile([128, N], F16, tag="ib")
            nc.sync.dma_start(out=ib[:], in_=ib_d)
            cwt = const.tile([128, L * 128], F16, tag="cwt")
            nc.sync.dma_start(out=cwt[:], in_=cwt_d.rearrange("p l o -> p (l o)"))
            w03 = const.tile([128, 128], F16, tag="w03")
            nc.sync.dma_start(out=w03[:], in_=w03_d)
            fc1w = const.tile([128, 128], F16, tag="fc1w")
            nc.sync.dma_start(out=fc1w[:], in_=fc1w_d)
            fc2w = const.tile([128, 1], F16, tag="fc2w")
            nc.sync.dma_start(out=fc2w[:], in_=fc2w_d)
            cbrep = const.tile([128, L * 512], F16, tag="cbrep")
            nc.sync.dma_start(out=cbrep[:], in_=cbr_d)
            fc1b = const.tile([128, 1], F32, tag="fc1b")
            nc.sync.dma_start(out=fc1b[:], in_=fc1b_d)
            fc2b = const.tile([128, 1], F32, tag="fc2b")
            nc.sync.dma_start(out=fc2b[:], in_=fc2b_d)
            idn = const.tile([64, 64], F16, tag="idn")
            nc.sync.dma_start(out=idn[:], in_=idn_d)
            xt = const.tile([128, N], F16, tag="xt")
            nc.sync.dma_start(out=xt[:], in_=xt_d)

            # persistent per-pair activations [128, N] fp16
            h = [hpool.tile([128, N], F16, tag=f"h{p}", name=f"h{p}")
                 for p in range(NP)]

            # ---- fc0 (block-diag K=6, bias via ones rows in xt) ----
            for p in range(NP):
                for g in range(NGRP):
                    zp = psz.tile([128, GRP], F32, tag="z")
                    gw = slice(g * GRP, (g + 1) * GRP)
                    for c2 in range(GRP // CH):
                        c0 = g * GRP + c2 * CH
                        sl = slice(c2 * CH, (c2 + 1) * CH)
                        nc.tensor.matmul(zp[:, sl], w03[32 * p:32 * p + 6, :],
                                         xt[32 * p:32 * p + 6, c0:c0 + CH],
                                         start=True, stop=True,
                                         tile_position=(32 * p, 0))
                    nc.scalar.activation(h[p][:, gw], zp[:], COPY)

            # ---- FNO layers ----
            for l in range(L):
                wml = wmp.tile([64, MODES * 3 * 64], F16, tag="wml")
                nc.sync.dma_start(out=wml[:], in_=wm_d[:, l].rearrange("p b c d -> p (b c d)"))

                # transpose h slices into hta (cols = (a, pair, c)), then FT
                hta = htp.tile([128, NA * 512], F16, tag="hta", name=f"hta{l}")
                hta4 = hta[:].rearrange("p (a e c) -> p a e c", a=NA, e=NP)
                for p in range(NP):
                    for q in range(NSL):
                        nc.sync.dma_start(
                            out=hta4[:, q * APS:(q + 1) * APS, p, :],
                            in_=h[p][:, q * SL:(q + 1) * SL], transpose=True)
                ftp = ps.tile([K2, 4 * 128], F32, tag="ft")
                for a in range(NA):
                    nc.tensor.matmul(ftp[:], fb[:, a * K2:(a + 1) * K2],
                                     hta[:, a * 512:(a + 1) * 512],
                                     start=(a == 0), stop=(a == NA - 1))
                ftsb = small.tile([K2, 512], F16, tag="ftsb")
                nc.vector.tensor_copy(ftsb[:], ftp[:])

                # FTI[i, 32 e + kk] via 2 stream transposes (i-halves)
                fti = small.tile([64, 256], F16, tag="fti")
                ftsb3 = ftsb[:].rearrange("p (e c) -> p e c", e=8)
                for hh in range(2):
                    nc.vector.transpose(fti[32 * hh:32 * hh + 32, :],
                                        ftsb3[:, :, 32 * hh:32 * hh + 32])

                # mode mix -> mmp[o, 8*(2k+s') + e]
                mmp = ps.tile([64, 256], F32, tag="mm")
                for k in range(MODES):
                    wb = (k * 3) * 64
                    wr = wml[:, wb:wb + 64]
                    wi = wml[:, wb + 64:wb + 128]
                    nwr = wml[:, wb + 128:wb + 192]
                    ftc = fti[:, 2 * k::K2]
                    fts = fti[:, 2 * k + 1::K2]
                    nc.tensor.matmul(mmp[:, 16 * k:16 * k + 8], wr, ftc,
                                     start=True, stop=False)
                    nc.tensor.matmul(mmp[:, 16 * k:16 * k + 8], wi, fts,
                                     start=False, stop=True)
                    nc.tensor.matmul(mmp[:, 16 * k + 8:16 * k + 16], wi, ftc,
                                     start=True, stop=False)
                    nc.tensor.matmul(mmp[:, 16 * k + 8:16 * k + 16], nwr, fts,
                                     start=False, stop=True)
                mmsb = small.tile([64, 256], F16, tag="mmsb")
                nc.vector.tensor_copy(mmsb[:], mmp[:])

                # MT[band + kk, 64 e + o] via 16 PE transposes; row 32/96 = cb
                mtp = ps.tile([128, 512], F16, tag="mtops")
                for e in range(8):
                    src = mmsb[:, e::8]   # [64 o, 32 kk]
                    for band in range(2):
                        nc.tensor.transpose(mtp[64 * band:64 * band + 32, 64 * e:64 * e + 64],
                                            src, idn[:], tile_position=(0, 64 * band))
                mt = small.tile([128, 512], F16, tag="mtsb")
                nc.vector.tensor_copy(mt[:], mtp[:])
                nc.vector.tensor_copy(mt[32:33, :], cbrep[32:33, l * 512:(l + 1) * 512])
                nc.vector.tensor_copy(mt[96:97, :], cbrep[96:97, l * 512:(l + 1) * 512])

                # inverse FT (K=33 incl. bias row) + pointwise (block-diag) + act
                act = GELU if l < L - 1 else COPY
                for p in range(NP):
                    for g in range(NGRP):
                        zp = psz.tile([128, GRP], F32, tag="z")
                        for c2 in range(GRP // CH):
                            c0 = g * GRP + c2 * CH
                            sl = slice(c2 * CH, (c2 + 1) * CH)
                            nc.tensor.matmul(zp[:, sl], cwt[:, l * 128:(l + 1) * 128],
                                             h[p][:, c0:c0 + CH], start=True, stop=False)
                        for c2 in range(GRP // CH):
                            c0 = g * GRP + c2 * CH
                            sl = slice(c2 * CH, (c2 + 1) * CH)
                            nc.tensor.matmul(zp[0:64, sl],
                                             mt[0:33, (2 * p) * 64:(2 * p) * 64 + 64],
                                             ib[0:33, c0:c0 + CH], start=False, stop=False,
                                             tile_position=(0, 0))
                            nc.tensor.matmul(zp[64:128, sl],
                                             mt[64:97, (2 * p + 1) * 64:(2 * p + 1) * 64 + 64],
                                             ib[64:97, c0:c0 + CH], start=False, stop=True,
                                             tile_position=(64, 64))
                        nc.scalar.activation(h[p][:, g * GRP:(g + 1) * GRP], zp[:], act)

            # ---- fc1 -> gelu -> fc2 -> out ----
            for p in range(NP):
                for g in range(NGRP):
                    op = ps.tile([128, GRP], F32, tag="mtops")
                    gsbs = []
                    for e2 in range(2):
                        gp = psz.tile([128, GRP], F32, tag="z")
                        for c2 in range(GRP // CH):
                            c0 = g * GRP + c2 * CH
                            nc.tensor.matmul(gp[:, c2 * CH:(c2 + 1) * CH],
                                             fc1w[64 * e2:64 * e2 + 64, :],
                                             h[p][64 * e2:64 * e2 + 64, c0:c0 + CH],
                                             start=True, stop=True)
                        gsb = gpool.tile([128, GRP], F16, tag=f"gsb{e2}", name=f"gsb{e2}")
                        nc.scalar.activation(gsb[:], gp[:], GELU, bias=fc1b[:])
                        gsbs.append(gsb)
                    for c2 in range(GRP // CH):
                        sl = slice(c2 * CH, (c2 + 1) * CH)
                        for e2 in range(2):
                            nc.tensor.matmul(op[32 * e2:32 * e2 + 1, sl], fc2w[:],
                                             gsbs[e2][:, sl], start=True, stop=True,
                                             tile_position=(0, 32 * e2))
                    osb = opool.tile([64, GRP], F32, tag="osb")
                    nc.vector.tensor_scalar_add(osb[:], op[0:64, :], fc2b[0:64, :])
                    for e2 in range(2):
                        e = 2 * p + e2
                        nc.sync.dma_start(out=out_d[e:e + 1, g * GRP:(g + 1) * GRP],
                                          in_=osb[32 * e2:32 * e2 + 1, :])

    nc.compile()
    _cache["nc"] = nc
    return nc


def _consts(fc0_w, fc0_b, sw_r, sw_i, cw, cb, fc1_w, fc1_b, fc2_w, fc2_b):
    f16 = np.float16
    n = np.arange(N)
    k = np.arange(MODES)
    ang = 2.0 * np.pi * np.outer(n, k) / N            # [N, MODES]
    Fb = np.empty((N, K2), np.float32)
    Fb[:, 0::2] = np.cos(ang)
    Fb[:, 1::2] = np.sin(ang)
    fb = np.ascontiguousarray(
        Fb.reshape(NA, 128, K2).transpose(1, 0, 2)).astype(f16)

    alpha = np.where(k == 0, 1.0, 2.0)
    IBr = np.empty((K2, N), np.float32)
    IBr[0::2, :] = (alpha[:, None] * np.cos(ang.T)) / N
    IBr[1::2, :] = -2.0 * np.sin(ang.T) / N
    ib = np.zeros((128, N), np.float32)
    ib[0:K2] = IBr
    ib[64:64 + K2] = IBr
    ib[K2, :] = 1.0
    ib[64 + K2, :] = 1.0
    ib = ib.astype(f16)

    wm = np.empty((64, L, MODES, 3, 64), np.float32)
    for l in range(L):
        for kk in range(MODES):
            wm[:, l, kk, 0, :] = sw_r[l, :, :, kk]
            wm[:, l, kk, 1, :] = sw_i[l, :, :, kk]
            wm[:, l, kk, 2, :] = -sw_r[l, :, :, kk]
    wm = wm.astype(f16)

    cwt = np.zeros((128, L, 128), np.float32)
    for l in range(L):
        cwt[0:64, l, 0:64] = cw[l].T
        cwt[64:128, l, 64:128] = cw[l].T

    cbrep = np.zeros((128, L * 512), np.float32)
    for l in range(L):
        cbrep[K2, l * 512:(l + 1) * 512] = np.tile(cb[l], 8)
        cbrep[64 + K2, l * 512:(l + 1) * 512] = np.tile(cb[l], 8)

    w03s = np.concatenate([fc0_w, fc0_b[None, :]], axis=0).astype(np.float32)
    w03 = np.zeros((128, 128), np.float32)
    for p in range(NP):
        w03[32 * p:32 * p + 3, 0:64] = w03s
        w03[32 * p + 3:32 * p + 6, 64:128] = w03s
    return dict(
        fb=fb, ib=ib, wm=wm,
        cwt=cwt.astype(f16), cbrep=cbrep.astype(f16), w03=w03.astype(f16),
        fc1w=np.concatenate([fc1_w, fc1_w], axis=0).astype(f16),
        fc2w=fc2_w.astype(f16),
        fc1b=fc1_b.astype(np.float32).reshape(128, 1),
        fc2b=np.full((128, 1), np.float32(np.asarray(fc2_b).reshape(-1)[0])),
        idn=np.eye(64, dtype=f16),
    )


def kernel(x, t, fc0_w, fc0_b, sw_r, sw_i, cw, cb, fc1_w, fc1_b, fc2_w, fc2_b,
           _trace=False, _tmpdir=None):
    nc = _build()
    consts = _consts(np.asarray(fc0_w), np.asarray(fc0_b), np.asarray(sw_r),
                     np.asarray(sw_i), np.asarray(cw), np.asarray(cb),
                     np.asarray(fc1_w), np.asarray(fc1_b), np.asarray(fc2_w),
                     np.asarray(fc2_b))
    x = np.asarray(x, np.float32).reshape(B, N).astype(np.float16)
    t = np.asarray(t, np.float32).reshape(B, N).astype(np.float16)
    in_maps = []
    for c in range(NCORES):
        m = dict(consts)
        xt = np.ones((128, N), np.float16)
        for p in range(NP):
            eA, eB = c * E + 2 * p, c * E + 2 * p + 1
            xt[32 * p + 0] = x[eA]
            xt[32 * p + 1] = t[eA]
            xt[32 * p + 3] = x[eB]
            xt[32 * p + 4] = t[eB]
        m["xt"] = xt
        in_maps.append(m)
    res = run_bass_kernel_spmd(nc, in_maps, list(range(NCORES)),
                               trace=_trace, tmpdir=_tmpdir)
    out = np.concatenate([res.results[c]["out"] for c in range(NCORES)], axis=0)
    kernel.last_result = res
    return out.reshape(B, N, 1).astype(np.float32)


# revision 12
# speedup vs baseline: 1.4707x; 1.0884x over previous
"""Trainium2 Bass kernel for FNO1d (B=64, N=8192, W=64, MODES=16, 4 layers).

Pure data-parallel over batch: 8 elements per NeuronCore on 8 cores. Per core,
elements form 4 pairs (2x64 channels -> 128 partitions). rfft/irfft are
replaced by 16-mode DFT matmuls. All activations/weights are fp16 in SBUF
(accumulation in fp32 PSUM); biases are folded into matmuls (fc0 via a ones
row, layer cb via an extra ones-row of the inverse basis).
"""
import sys
import numpy as np

sys.path.insert(0, "/opt/trn_rl_repo")

import concourse.bacc as bacc
import concourse.mybir as mybir
import concourse.tile as tile
from concourse.bass_utils import run_bass_kernel_spmd

F16 = mybir.dt.float16
F32 = mybir.dt.float32
GELU = mybir.ActivationFunctionType.Gelu
COPY = mybir.ActivationFunctionType.Copy

B, N, W, MODES, L = 64, 8192, 64, 16, 4
NCORES = 8
E = B // NCORES          # 8 elems per core
NP = E // 2              # 4 pairs
K2 = 2 * MODES           # 32 interleaved (cos,sin) rows
CH = 512
GRP = 1024
NGRP = N // GRP          # 8
NA = N // 128            # 64 transpose chunks
SL = 2048                # transpose slice width
NSL = N // SL            # 4 slices
APS = SL // 128          # a-chunks per slice = 16

_cache = {}


def _build():
    if "nc" in _cache:
        return _cache["nc"]
    nc = bacc.Bacc("TRN2", target_bir_lowering=False, debug=False,
                   num_devices=NCORES)

    def din(name, shape, dt):
        return nc.dram_tensor(name, shape, dt, kind="ExternalInput").ap()

    xt_d = din("xt", [128, N], F16)      # rows 32p..32p+6 = (xA, tA, 1, xB, tB, 1)
    fb_d = din("fb", [128, NA, K2], F16)         # fb[p,a,kk] = Fb[128a+p, kk]
    ib_d = din("ib", [128, N], F16)              # rows 0:33 & 64:97 = [IB; ones]
    wm_d = din("wm", [64, L, MODES, 3, 64], F16)  # (i, l, k, {wr, wi, -wr}, o)
    cwt_d = din("cwt", [128, L, 128], F16)       # block-diag [cw.T 0; 0 cw.T]
    w03_d = din("w03", [2, 64], F16)             # (w0; w1) for spectral outer
    w1b_d = din("w1blk", [128, 128], F16)        # layer0 v: rank-3 block at rows 32p
    fc1w_d = din("fc1w", [128, 128], F16)        # both bands = fc1_w
    fc2w_d = din("fc2w", [128, 1], F16)
    cbr_d = din("cbrep", [128, L * 512], F16)    # rows 32/96: tile(cb[l], 8)
    fc1b_d = din("fc1b", [128, 1], F32)
    fc2b_d = din("fc2b", [128, 1], F32)
    idn_d = din("idn", [64, 64], F16)
    out_d = nc.dram_tensor("out", [E, N], F32, kind="ExternalOutput").ap()

    with tile.TileContext(nc) as tc:
        import contextlib
        with contextlib.ExitStack() as ctx:
            const = ctx.enter_context(tc.tile_pool(name="const", bufs=1))
            hpool = ctx.enter_context(tc.tile_pool(name="h", bufs=1))
            htp = ctx.enter_context(tc.tile_pool(name="htp", bufs=1))
            small = ctx.enter_context(tc.tile_pool(name="small", bufs=2))
            wmp = ctx.enter_context(tc.tile_pool(name="wmp", bufs=1))
            gpool = ctx.enter_context(tc.tile_pool(name="g", bufs=2))
            opool = ctx.enter_context(tc.tile_pool(name="o", bufs=2))
            ps = ctx.enter_context(tc.tile_pool(name="ps", bufs=1, space="PSUM"))
            psz = ctx.enter_context(tc.tile_pool(name="psz", bufs=2, space="PSUM"))

            # ---- constants ----
            fb = const.tile([128, NA * K2], F16, tag="fb")
            nc.sync.dma_start(out=fb[:], in_=fb_d.rearrange("p a k -> p (a k)"))
            ib = const.tile([128, N], F16, tag="ib")
            nc.sync.dma_start(out=ib[:], in_=ib_d)
            cwt = const.tile([128, L * 128], F16, tag="cwt")
            nc.sync.dma_start(out=cwt[:], in_=cwt_d.rearrange("p l o -> p (l o)"))
            w03 = const.tile([2, 64], F16, tag="w03")
            nc.sync.dma_start(out=w03[:], in_=w03_d)
            w1blk = const.tile([128, 128], F16, tag="w1blk")
            nc.sync.dma_start(out=w1blk[:], in_=w1b_d)
            fc1w = const.tile([128, 128], F16, tag="fc1w")
            nc.sync.dma_start(out=fc1w[:], in_=fc1w_d)
            fc2w = const.tile([128, 1], F16, tag="fc2w")
            nc.sync.dma_start(out=fc2w[:], in_=fc2w_d)
            cbrep = const.tile([128, L * 512], F16, tag="cbrep")
            nc.sync.dma_start(out=cbrep[:], in_=cbr_d)
            fc1b = const.tile([128, 1], F32, tag="fc1b")
            nc.sync.dma_start(out=fc1b[:], in_=fc1b_d)
            fc2b = const.tile([128, 1], F32, tag="fc2b")
            nc.sync.dma_start(out=fc2b[:], in_=fc2b_d)
            idn = const.tile([64, 64], F16, tag="idn")
            nc.sync.dma_start(out=idn[:], in_=idn_d)
            xt = const.tile([128, N], F16, tag="xt")
            nc.sync.dma_start(out=xt[:], in_=xt_d)

            # persistent per-pair activations [128, N] fp16
            h = [hpool.tile([128, N], F16, tag=f"h{p}", name=f"h{p}")
                 for p in range(NP)]


            # ---- FNO layers ----
            for l in range(L):
                wml = wmp.tile([64, MODES * 3 * 64], F16, tag="wml")
                nc.sync.dma_start(out=wml[:], in_=wm_d[:, l].rearrange("p b c d -> p (b c d)"))

                ftp = ps.tile([K2, 4 * 128], F32, tag="ft")
                if l == 0:
                    # spectral shortcut: ft(h0) from x/t spectra (h0 never built)
                    xtt = htp.tile([128, NA * 128], F16, tag="hta", name="xtt")
                    xtt4 = xtt[:].rearrange("p (a c) -> p a c", a=NA)
                    for p in range(NP):
                        nc.sync.dma_start(out=xtt4[:, :, 32 * p:32 * p + 32],
                                          in_=xt[32 * p:32 * p + 32, :],
                                          transpose=True)
                    fxp = ps.tile([K2, 256], F32, tag="mm")
                    for a in range(NA):
                        nc.tensor.matmul(fxp[:, 0:128], fb[:, a * K2:(a + 1) * K2],
                                         xtt[:, a * 128:(a + 1) * 128],
                                         start=(a == 0), stop=(a == NA - 1))
                    fxs = small.tile([K2, 128], F16, tag="fxs")
                    nc.vector.tensor_copy(fxs[:], fxp[:, 0:128])
                    # per elem: transpose (x,t) spectra -> [2, 32]
                    ftx = ps.tile([2, 256], F16, tag="mtops")
                    for p in range(NP):
                        for e2 in range(2):
                            e = 2 * p + e2
                            cc = 32 * p + 3 * e2
                            nc.tensor.transpose(ftx[0:2, 32 * e:32 * e + 32],
                                                fxs[:, cc:cc + 2], idn[0:K2, 0:K2])
                    fxt = small.tile([2, 256], F16, tag="fxt")
                    nc.vector.tensor_copy(fxt[:], ftx[:])
                    # outer product: ftp[:, 64 e:64 e+64] = fxt_e.T @ (w0; w1)
                    for e in range(8):
                        nc.tensor.matmul(ftp[:, 64 * e:64 * e + 64],
                                         fxt[0:2, 32 * e:32 * e + 32],
                                         w03[0:2, 0:64], start=True, stop=True)
                else:
                    # transpose h slices into hta (cols = (a, pair, c)), then FT
                    hta = htp.tile([128, NA * 512], F16, tag="hta", name=f"hta{l}")
                    hta4 = hta[:].rearrange("p (a e c) -> p a e c", a=NA, e=NP)
                    for p in range(NP):
                        for q in range(NSL):
                            nc.sync.dma_start(
                                out=hta4[:, q * APS:(q + 1) * APS, p, :],
                                in_=h[p][:, q * SL:(q + 1) * SL], transpose=True)
                    for a in range(NA):
                        nc.tensor.matmul(ftp[:], fb[:, a * K2:(a + 1) * K2],
                                         hta[:, a * 512:(a + 1) * 512],
                                         start=(a == 0), stop=(a == NA - 1))
                ftsb = small.tile([K2, 512], F16, tag="ftsb")
                nc.vector.tensor_copy(ftsb[:], ftp[:])

                # FTI[i, 32 e + kk] via 2 stream transposes (i-halves)
                fti = small.tile([64, 256], F16, tag="fti")
                ftsb3 = ftsb[:].rearrange("p (e c) -> p e c", e=8)
                for hh in range(2):
                    nc.vector.transpose(fti[32 * hh:32 * hh + 32, :],
                                        ftsb3[:, :, 32 * hh:32 * hh + 32])

                # mode mix -> mmp[o, 8*(2k+s') + e]
                mmp = ps.tile([64, 256], F32, tag="mm")
                for k in range(MODES):
                    wb = (k * 3) * 64
                    wr = wml[:, wb:wb + 64]
                    wi = wml[:, wb + 64:wb + 128]
                    nwr = wml[:, wb + 128:wb + 192]
                    ftc = fti[:, 2 * k::K2]
                    fts = fti[:, 2 * k + 1::K2]
                    nc.tensor.matmul(mmp[:, 16 * k:16 * k + 8], wr, ftc,
                                     start=True, stop=False)
                    nc.tensor.matmul(mmp[:, 16 * k:16 * k + 8], wi, fts,
                                     start=False, stop=True)
                    nc.tensor.matmul(mmp[:, 16 * k + 8:16 * k + 16], wi, ftc,
                                     start=True, stop=False)
                    nc.tensor.matmul(mmp[:, 16 * k + 8:16 * k + 16], nwr, fts,
                                     start=False, stop=True)
                mmsb = small.tile([64, 256], F16, tag="mmsb")
                nc.vector.tensor_copy(mmsb[:], mmp[:])

                # MT[kk, 64 e + o] via 8 PE transposes; row 32 = cb bias
                mtp = ps.tile([128, 512], F16, tag="mtops")
                for e in range(8):
                    src = mmsb[:, e::8]   # [64 o, 32 kk]
                    nc.tensor.transpose(mtp[0:32, 64 * e:64 * e + 64], src, idn[:])
                mt = small.tile([64, 512], F16, tag="mtsb")
                nc.vector.tensor_copy(mt[0:32, :], mtp[0:32, :])
                nc.vector.tensor_copy(mt[32:33, :], cbrep[32:33, l * 512:(l + 1) * 512])

                # inverse FT (K=33 incl. bias row) + pointwise (block-diag) + act
                act = GELU if l < L - 1 else COPY
                for p in range(NP):
                    for g in range(NGRP):
                        zp = psz.tile([128, GRP], F32, tag="z")
                        for c2 in range(GRP // CH):
                            c0 = g * GRP + c2 * CH
                            sl = slice(c2 * CH, (c2 + 1) * CH)
                            if l == 0:
                                nc.tensor.matmul(zp[:, sl], w1blk[32 * p:32 * p + 6, :],
                                                 xt[32 * p:32 * p + 6, c0:c0 + CH],
                                                 start=True, stop=False,
                                                 tile_position=(32 * p, 0))
                            else:
                                nc.tensor.matmul(zp[:, sl], cwt[:, l * 128:(l + 1) * 128],
                                                 h[p][:, c0:c0 + CH], start=True, stop=False)
                        for c2 in range(GRP // CH):
                            c0 = g * GRP + c2 * CH
                            sl = slice(c2 * CH, (c2 + 1) * CH)
                            nc.tensor.matmul(zp[:, sl],
                                             mt[0:33, 128 * p:128 * p + 128],
                                             ib[0:33, c0:c0 + CH], start=False, stop=True)
                        nc.scalar.activation(h[p][:, g * GRP:(g + 1) * GRP], zp[:], act)

            # ---- fc1 -> gelu -> fc2 -> out ----
            for p in range(NP):
                for g in range(NGRP):
                    op = ps.tile([128, GRP], F32, tag="mtops")
                    gsbs = []
                    for e2 in range(2):
                        gp = psz.tile([128, GRP], F32, tag="z")
                        for c2 in range(GRP // CH):
                            c0 = g * GRP + c2 * CH
                            nc.tensor.matmul(gp[:, c2 * CH:(c2 + 1) * CH],
                                             fc1w[64 * e2:64 * e2 + 64, :],
                                             h[p][64 * e2:64 * e2 + 64, c0:c0 + CH],
                                             start=True, stop=True)
                        gsb = gpool.tile([128, GRP], F16, tag=f"gsb{e2}", name=f"gsb{e2}")
                        nc.scalar.activation(gsb[:], gp[:], GELU, bias=fc1b[:])
                        gsbs.append(gsb)
                    for c2 in range(GRP // CH):
                        sl = slice(c2 * CH, (c2 + 1) * CH)
                        for e2 in range(2):
                            nc.tensor.matmul(op[32 * e2:32 * e2 + 1, sl], fc2w[:],
                                             gsbs[e2][:, sl], start=True, stop=True,
                                             tile_position=(0, 32 * e2))
                    osb = opool.tile([64, GRP], F32, tag="osb")
                    nc.vector.tensor_scalar_add(osb[:], op[0:64, :], fc2b[0:64, :])
                    for e2 in range(2):
                        e = 2 * p + e2
                        nc.sync.dma_start(out=out_d[e:e + 1, g * GRP:(g + 1) * GRP],
                                          in_=osb[32 * e2:32 * e2 + 1, :])

    nc.compile()
    _cache["nc"] = nc
    return nc


def _consts(fc0_w, fc0_b, sw_r, sw_i, cw, cb, fc1_w, fc1_b, fc2_w, fc2_b):
    f16 = np.float16
    n = np.arange(N)
    k = np.arange(MODES)
    ang = 2.0 * np.pi * np.outer(n, k) / N            # [N, MODES]
    Fb = np.empty((N, K2), np.float32)
    Fb[:, 0::2] = np.cos(ang)
    Fb[:, 1::2] = np.sin(ang)
    fb = np.ascontiguousarray(
        Fb.reshape(NA, 128, K2).transpose(1, 0, 2)).astype(f16)

    alpha = np.where(k == 0, 1.0, 2.0)
    IBr = np.empty((K2, N), np.float32)
    IBr[0::2, :] = (alpha[:, None] * np.cos(ang.T)) / N
    IBr[1::2, :] = -2.0 * np.sin(ang.T) / N
    ib = np.zeros((128, N), np.float32)
    ib[0:K2] = IBr
    ib[K2, :] = 1.0
    ib = ib.astype(f16)

    wm = np.empty((64, L, MODES, 3, 64), np.float32)
    for l in range(L):
        for kk in range(MODES):
            wm[:, l, kk, 0, :] = sw_r[l, :, :, kk]
            wm[:, l, kk, 1, :] = sw_i[l, :, :, kk]
            wm[:, l, kk, 2, :] = -sw_r[l, :, :, kk]
    wm = wm.astype(f16)

    cwt = np.zeros((128, L, 128), np.float32)
    for l in range(L):
        cwt[0:64, l, 0:64] = cw[l].T
        cwt[64:128, l, 64:128] = cw[l].T

    cbrep = np.zeros((128, L * 512), np.float32)
    for l in range(L):
        cbl = cb[l].astype(np.float32).copy()
        if l == 0:
            # DC correction: spectral path drops sum_n(b) term; fold into bias
            cbl = cbl + sw_r[0, :, :, 0].T @ fc0_b
        cbrep[K2, l * 512:(l + 1) * 512] = np.tile(cbl, 8)

    w03 = fc0_w.astype(np.float32)                      # [2, 64]
    u = np.stack([cw[0] @ fc0_w[0], cw[0] @ fc0_w[1], cw[0] @ fc0_b], axis=0)
    w1blk = np.zeros((128, 128), np.float32)
    for p in range(NP):
        w1blk[32 * p:32 * p + 3, 0:64] = u
        w1blk[32 * p + 3:32 * p + 6, 64:128] = u
    return dict(
        fb=fb, ib=ib, wm=wm,
        cwt=cwt.astype(f16), cbrep=cbrep.astype(f16), w03=w03.astype(f16),
        w1blk=w1blk.astype(f16),
        fc1w=np.concatenate([fc1_w, fc1_w], axis=0).astype(f16),
        fc2w=fc2_w.astype(f16),
        fc1b=fc1_b.astype(np.float32).reshape(128, 1),
        fc2b=np.full((128, 1), np.float32(np.asarray(fc2_b).reshape(-1)[0])),
        idn=np.eye(64, dtype=f16),
    )


def kernel(x, t, fc0_w, fc0_b, sw_r, sw_i, cw, cb, fc1_w, fc1_b, fc2_w, fc2_b,
           _trace=False, _tmpdir=None):
    nc = _build()
    consts = _consts(np.asarray(fc0_w), np.asarray(fc0_b), np.asarray(sw_r),
                     np.asarray(sw_i), np.asarray(cw), np.asarray(cb),
                     np.asarray(fc1_w), np.asarray(fc1_b), np.asarray(fc2_w),
                     np.asarray(fc2_b))
    x = np.asarray(x, np.float32).reshape(B, N).astype(np.float16)
    t = np.asarray(t, np.float32).reshape(B, N).astype(np.float16)
    in_maps = []
    for c in range(NCORES):
        m = dict(consts)
        xt = np.ones((128, N), np.float16)
        for p in range(NP):
            eA, eB = c * E + 2 * p, c * E + 2 * p + 1
            xt[32 * p + 0] = x[eA]
            xt[32 * p + 1] = t[eA]
            xt[32 * p + 3] = x[eB]
            xt[32 * p + 4] = t[eB]
        m["xt"] = xt
        in_maps.append(m)
    res = run_bass_kernel_spmd(nc, in_maps, list(range(NCORES)),
                               trace=_trace, tmpdir=_tmpdir)
    out = np.concatenate([res.results[c]["out"] for c in range(NCORES)], axis=0)
    kernel.last_result = res
    return out.reshape(B, N, 1).astype(np.float32)
